# revision 7
# baseline (speedup 1.0000x reference)
# Multi-head causal attention (B=4, S=2048, D=1024, H=16) on 8 TRN2 NeuronCores.
#
# Sharding: batch x query-chunk at 256-row granularity. Core c handles batch
# b=c//2; its parity picks four 256-row query chunks balanced so both
# parities fit uniform kk-tile slot capacities (16,12,8,4):
#   parity 0 -> chunks [7,4,3,0] (needs 16,10,8,2)
#   parity 1 -> chunks [6,5,2,1] (needs 14,12,6,4)
# The SPMD program is identical on every core; per-core causality/padding is
# expressed via additive score masks (0 / -30000) in input data.
#
# Inner machinery:
#   St[kk, 4 heads x 256q] = Kt_tile.T @ Qt  (4 MMs, 2 concurrent pairs via
#     tile_position row quadrants), fp32 PSUM.
#   DVE drains PSUM -> SBUF bf16, fusing the additive causal mask.
#   ACT exp runs on batched [128, 4096] SBUF spans (4 kk-tiles at once).
#   OT[dv, q] += V_aug[kk, 65].T @ P  -- V carries a ones column so PSUM row
#     64 accumulates softmax denominators for free.
#   Output projection is interleaved between attention blocks (fills the PE
#   while ACT is busy) and y is written out in bf16 across two DMA queues.
import sys

if '/opt/trn_rl_repo' not in sys.path:
    sys.path.insert(0, '/opt/trn_rl_repo')

import numpy as np

B, S, D = 4, 2048, 1024
H, DK = 16, 64
NCORES = 8
SC2 = 256                  # query chunk rows
NKT = S // 128             # 16 kk tiles
HPN = D // 128             # 8 head-pairs
CAPS4 = (16, 12, 8, 4)     # kk-tile capacity per slot (uniform across cores)
CHUNKS_P = {0: (7, 4, 3, 0), 1: (6, 5, 2, 1)}  # chunk per slot, by parity

_CACHE = {}


def _build_program():
    import contextlib

    import concourse.tile as tile
    from concourse import bacc, mybir

    F32 = mybir.dt.float32
    BF16 = mybir.dt.bfloat16
    EXP = mybir.ActivationFunctionType.Exp

    nc = bacc.Bacc("TRN2", target_bir_lowering=False, debug=False,
                   num_devices=NCORES)

    xT_d = nc.dram_tensor("xT", [D, S], BF16, kind="ExternalInput")
    xQT_d = nc.dram_tensor("xQT", [D, 4 * SC2], BF16, kind="ExternalInput")
    wqT_d = nc.dram_tensor("wqT", [D, D], BF16, kind="ExternalInput")
    wkT_d = nc.dram_tensor("wkT", [D, D], BF16, kind="ExternalInput")
    wvT_d = nc.dram_tensor("wvT", [D, D], BF16, kind="ExternalInput")
    woT_d = nc.dram_tensor("woT", [D, D], BF16, kind="ExternalInput")
    bias_d = nc.dram_tensor("bias", [1, D], BF16, kind="ExternalInput")
    masks_d = nc.dram_tensor("masks", [128, 16 * 512], BF16,
                             kind="ExternalInput")
    y_d = nc.dram_tensor("y", [4 * SC2, D], BF16, kind="ExternalOutput")

    with tile.TileContext(nc) as tc, contextlib.ExitStack() as ctx:
        smalls = ctx.enter_context(tc.tile_pool(name="smalls", bufs=1))
        p_OT = ctx.enter_context(tc.tile_pool(name="otp", bufs=1))
        p_Kt = ctx.enter_context(tc.tile_pool(name="ktp", bufs=1))
        p_Qt = ctx.enter_context(tc.tile_pool(name="qtp", bufs=1))
        p_V = ctx.enter_context(tc.tile_pool(name="vp", bufs=1))
        p_mk = ctx.enter_context(tc.tile_pool(name="mk", bufs=1))

        OT = p_OT.tile([128, HPN * 4 * SC2], BF16, tag="OT")
        Kt = p_Kt.tile([128, HPN * S], BF16, tag="Kt")
        Qt = p_Qt.tile([128, HPN * 4 * SC2], BF16, tag="Qt")
        Vsb = p_V.tile([128, NKT * H * 65], BF16, tag="Vsb")
        masks_sb = p_mk.tile([128, 16 * 512], BF16, tag="masks")
        bias_sb = smalls.tile([1, D], BF16, tag="bias")
        ones1f = smalls.tile([1, 128], F32, tag="ones1f")
        nc.vector.memset(ones1f[:], 1.0)
        ones1 = smalls.tile([1, 128], BF16, tag="ones1")
        nc.vector.tensor_copy(ones1[:], ones1f[:])
        ones256f = smalls.tile([128, 256], F32, tag="ones256f")
        nc.vector.memset(ones256f[:], 1.0)

        # ones columns of V_aug (all 16 s-tiles, one strided copy)
        nc.vector.tensor_copy(
            Vsb[:].rearrange("p (s h c) -> p s h c", s=NKT, c=65)
            [:, :, :, 64:65],
            ones256f[:].rearrange("p (s h) -> p s h", s=NKT)[:, :, :, None])

        # ---- V + K projections, one half of the sequence at a time ----
        with tc.tile_pool(name="xth", bufs=2) as p_xh, \
             tc.tile_pool(name="wfv", bufs=1) as p_wv, \
             tc.tile_pool(name="wfk", bufs=1) as p_wk, \
             tc.tile_pool(name="psp", bufs=8, space="PSUM") as psp:
            wv = p_wv.tile([128, 8 * D], BF16, tag="wv")
            wk = p_wk.tile([128, 8 * D], BF16, tag="wk")
            xhs = [p_xh.tile([128, 8 * 1024], BF16, tag="xh",
                             name=f"xh_{h}") for h in range(2)]
            # interleave so the first V matmul group's inputs land first
            for k in range(8):
                nc.sync.dma_start(
                    xhs[0][:, k * 1024:(k + 1) * 1024],
                    xT_d.ap()[k * 128:(k + 1) * 128, 0:1024])
                nc.sync.dma_start(
                    wv[:, k * D:(k + 1) * D],
                    wvT_d.ap()[k * 128:(k + 1) * 128, :])
            for k in range(8):
                nc.sync.dma_start(
                    wk[:, k * D:(k + 1) * D],
                    wkT_d.ap()[k * 128:(k + 1) * 128, :])
                nc.sync.dma_start(
                    xhs[1][:, k * 1024:(k + 1) * 1024],
                    xT_d.ap()[k * 128:(k + 1) * 128, 1024:2048])
            # late inputs (needed only from attention onward)
            nc.gpsimd.dma_start(masks_sb[:], masks_d.ap())
            nc.gpsimd.dma_start(bias_sb[:], bias_d.ap())

            for half in range(2):
                xh = xhs[half]
                # V for the 8 s-tiles of this half (into SBUF V_aug layout)
                for sti in range(8):
                    st_g = half * 8 + sti
                    for dvc in range(2):
                        ps = psp.tile([128, 512], F32, tag="ps")
                        for k in range(8):
                            nc.tensor.matmul(
                                ps[:],
                                xh[:, k * 1024 + sti * 128:
                                   k * 1024 + (sti + 1) * 128],
                                wv[:, k * D + dvc * 512:k * D + (dvc + 1) * 512],
                                start=(k == 0), stop=(k == 7))
                        off = st_g * 1040 + dvc * 520
                        nc.vector.tensor_copy(
                            Vsb[:, off:off + 520]
                            .rearrange("p (h c) -> p h c", c=65)[:, :, 0:64],
                            ps[:].rearrange("p (h c) -> p h c", c=64))
                # K for the 2 s-chunks of this half -> SBUF-resident Kt
                for sc2 in range(2):
                    sc = half * 2 + sc2
                    ps8 = [psp.tile([128, 512], F32, tag="ps",
                                    name=f"psk_{sc}_{hp}")
                           for hp in range(HPN)]
                    for k in range(8):
                        for hp in range(HPN):
                            nc.tensor.matmul(
                                ps8[hp][:],
                                wk[:, k * D + hp * 128:k * D + (hp + 1) * 128],
                                xh[:, k * 1024 + sc2 * 512:
                                   k * 1024 + (sc2 + 1) * 512],
                                start=(k == 0), stop=(k == 7))
                    for hp in range(HPN):
                        nc.vector.tensor_copy(
                            Kt[:, hp * S + sc * 512:hp * S + (sc + 1) * 512],
                            ps8[hp][:])

        # ------------- Q projection (xQT streamed, wq resident) ----------
        with tc.tile_pool(name="wf2", bufs=1) as p_w2, \
             tc.tile_pool(name="xqs", bufs=4) as p_xq, \
             tc.tile_pool(name="psq", bufs=8, space="PSUM") as psq:
            wq = p_w2.tile([128, 8 * D], BF16, tag="w2")
            for k in range(8):
                nc.sync.dma_start(
                    wq[:, k * D:(k + 1) * D],
                    wqT_d.ap()[k * 128:(k + 1) * 128, :])
            for ci in range(2):
                ps8 = [psq.tile([128, 512], F32, tag="ps",
                                name=f"psq_{ci}_{hp}") for hp in range(HPN)]
                for k in range(8):
                    xq1 = p_xq.tile([128, 512], BF16, tag="xq")
                    nc.sync.dma_start(
                        xq1[:],
                        xQT_d.ap()[k * 128:(k + 1) * 128,
                                   ci * 512:(ci + 1) * 512])
                    for hp in range(HPN):
                        nc.tensor.matmul(
                            ps8[hp][:],
                            wq[:, k * D + hp * 128:k * D + (hp + 1) * 128],
                            xq1[:], start=(k == 0), stop=(k == 7))
                for hp in range(HPN):
                    nc.vector.tensor_copy(
                        Qt[:, hp * 4 * SC2 + ci * 512:
                           hp * 4 * SC2 + (ci + 1) * 512],
                        ps8[hp][:])

        # ------------- attention + interleaved output projection ---------
        with tc.tile_pool(name="rs", bufs=2) as p_rs, \
             tc.tile_pool(name="bcp", bufs=2) as p_bc, \
             tc.tile_pool(name="stb", bufs=2) as p_stsb, \
             tc.tile_pool(name="pp", bufs=2) as p_P, \
             tc.tile_pool(name="wo", bufs=1) as p_wo, \
             tc.tile_pool(name="ybp", bufs=4) as p_yb, \
             tc.tile_pool(name="pst", bufs=2, space="PSUM") as p_st, \
             tc.tile_pool(name="pav", bufs=4, space="PSUM") as p_av:

            wo = p_wo.tile([128, 8 * D], BF16, tag="wo")
            for k in range(8):
                nc.sync.dma_start(
                    wo[:, k * D:(k + 1) * D],
                    woT_d.ap()[k * 128:(k + 1) * 128, :])

            # out-proj emitter for one (slot, half, nc2) unit
            def emit_outproj(s, h2, nc2, yq):
                qi = s * 2 + h2
                ps = p_av.tile([128, 512], F32, tag="av",
                               name=f"psy_{qi}_{nc2}")
                for dc in range(8):
                    nc.tensor.matmul(
                        ps[:],
                        OT[:, dc * 4 * SC2 + s * 256 + h2 * 128:
                           dc * 4 * SC2 + s * 256 + (h2 + 1) * 128],
                        wo[:, dc * D + nc2 * 512:dc * D + (nc2 + 1) * 512],
                        start=(dc == 0), stop=False)
                nc.tensor.matmul(
                    ps[:], ones1[:],
                    bias_sb[0:1, nc2 * 512:(nc2 + 1) * 512],
                    start=False, stop=True)
                yb = p_yb.tile([128, 512], BF16, tag="yb")
                nc.vector.tensor_copy(yb[:], ps[:])
                nc.sync.dma_start(
                    y_d.ap()[qi * 128:(qi + 1) * 128,
                             nc2 * 512:(nc2 + 1) * 512], yb[:])

            outproj_ready = []   # list of (s, h2, nc2) units ready to emit

            def drain_outproj(n):
                for _ in range(min(n, len(outproj_ready))):
                    emit_outproj(*outproj_ready.pop(0), None)

            # attention blocks: slots in increasing-cap order
            for s in (3, 2, 1, 0):
                cap = CAPS4[s]
                groups = cap // 4
                for hg in range(4):
                    av = [p_av.tile([128, 512], F32, tag="av",
                                    name=f"av_{s}_{hg}_{i}")
                          for i in range(2)]
                    Ps = []

                    def emit_scores(g, s=s, hg=hg, cap=cap, Ps=Ps):
                        stsb = p_stsb.tile([128, 4096], BF16, tag="stsb")
                        for tq in range(4):
                            t = g * 4 + tq
                            st = p_st.tile([128, 1024], F32, tag="st")
                            for hpi in range(2):
                                hp = 2 * hg + hpi
                                for hh in range(2):
                                    r0 = 64 * hh
                                    # concurrent hh=0/1 pair -> different
                                    # PSUM banks: col = hh*512 + hpi*256
                                    nc.tensor.matmul(
                                        st[:, hh * 512 + hpi * 256:
                                           hh * 512 + hpi * 256 + 256],
                                        Kt[r0:r0 + 64,
                                           hp * S + t * 128:
                                           hp * S + (t + 1) * 128],
                                        Qt[r0:r0 + 64,
                                           hp * 4 * SC2 + s * 256:
                                           hp * 4 * SC2 + (s + 1) * 256],
                                        start=True, stop=True,
                                        tile_position=(r0, 0))
                            if t >= cap - 4:
                                midx = s * 4 + (t - (cap - 4))
                                for hf in range(2):
                                    nc.vector.tensor_add(
                                        stsb[:, tq * 1024 + hf * 512:
                                             tq * 1024 + (hf + 1) * 512],
                                        st[:, hf * 512:(hf + 1) * 512],
                                        masks_sb[:, midx * 512:
                                                 (midx + 1) * 512])
                            else:
                                nc.vector.tensor_copy(
                                    stsb[:, tq * 1024:(tq + 1) * 1024], st[:])
                        P = p_P.tile([128, 4096], BF16, tag="p")
                        nc.scalar.activation(P[:], stsb[:], EXP)
                        Ps.append(P)

                    def emit_av(g, s=s, hg=hg, cap=cap, av=av, Ps=Ps):
                        P = Ps[g]
                        for tq in range(4):
                            t = g * 4 + tq
                            for hpi in range(2):
                                hp = 2 * hg + hpi
                                for hh in range(2):
                                    h = 2 * hpi + hh
                                    off = t * 1040 + hp * 130 + hh * 65
                                    # two heads share one PSUM bank: only the
                                    # first opens the accumulation group, only
                                    # the last closes it (per-element
                                    # has_written gives overwrite-then-add)
                                    nc.tensor.matmul(
                                        av[h // 2][0:65,
                                                   (h % 2) * 256:
                                                   (h % 2) * 256 + 256],
                                        Vsb[:, off:off + 65],
                                        P[:, tq * 1024 + hh * 512 + hpi * 256:
                                          tq * 1024 + hh * 512 + hpi * 256
                                          + 256],
                                        start=(t == 0 and h % 2 == 0),
                                        stop=(t == cap - 1 and h % 2 == 1))

                    # software pipeline: scores(g+1) overlaps exp(g); AV(g)
                    # follows so its P is ready by the time PE reaches it.
                    emit_scores(0)
                    for g in range(1, groups):
                        emit_scores(g)
                        emit_av(g - 1)
                        drain_outproj(1)
                    emit_av(groups - 1)

                    # normalize: softmax denominators sit in av row 64
                    rs = p_rs.tile([1, 1024], F32, tag="rs")
                    for h in range(4):
                        nc.vector.tensor_copy(
                            rs[0:1, h * 256:(h + 1) * 256],
                            av[h // 2][64:65, (h % 2) * 256:
                                       (h % 2) * 256 + 256])
                    bc = p_bc.tile([128, 1024], F32, tag="bc")
                    nc.gpsimd.partition_broadcast(bc[:], rs[:])
                    rbc = p_bc.tile([128, 1024], F32, tag="rbc")
                    scr = p_bc.tile([128, 1024], F32, tag="scr")
                    nc.vector.reciprocal_approx_accurate(
                        rbc[:], bc[:], scratch=scr[:])
                    for h in range(4):
                        hp = 2 * hg + h // 2
                        r0 = 64 * (h % 2)
                        nc.vector.tensor_mul(
                            OT[r0:r0 + 64,
                               hp * 4 * SC2 + s * 256:
                               hp * 4 * SC2 + (s + 1) * 256],
                            av[h // 2][0:64, (h % 2) * 256:
                                       (h % 2) * 256 + 256],
                            rbc[r0:r0 + 64, h * 256:(h + 1) * 256])
                # slot s fully normalized -> queue its out-proj units
                for h2 in range(2):
                    for nc2 in range(2):
                        outproj_ready.append((s, h2, nc2))
            drain_outproj(len(outproj_ready))

    nc.compile()
    return nc


def _get_program():
    if 'nc' not in _CACHE:
        _CACHE['nc'] = _build_program()
    return _CACHE['nc']


def _masks_for_core(c):
    """Additive masks [128, 16*512]: slot s, local index k (t = cap-4+k).
    Pattern [128, 512] = [A | B | A | B] where A/B are the two 128-query
    tiles of the chunk: 0 where key <= query else -30000."""
    import ml_dtypes
    par = c % 2
    out = np.zeros((128, 16 * 512), np.float32)
    p = np.arange(128)[:, None]
    f = np.arange(128)[None, :]
    for s in range(4):
        cap = CAPS4[s]
        j = CHUNKS_P[par][s]
        for k in range(4):
            t = cap - 4 + k
            blocks = []
            for half in range(2):
                attend = (t * 128 + p) <= (j * 256 + half * 128 + f)
                blocks.append(np.where(attend, 0.0, -30000.0))
            patt = np.hstack([blocks[0], blocks[1]])
            out[:, (s * 4 + k) * 512:(s * 4 + k) * 512 + 512] = \
                np.hstack([patt, patt])
    return out.astype(ml_dtypes.bfloat16)


def kernel(x, w_q, w_k, w_v, w_o, b_o):
    import ml_dtypes
    from concourse.bass_utils import run_bass_kernel_spmd

    BF = ml_dtypes.bfloat16
    x = np.asarray(x, dtype=np.float32)
    nc = _get_program()

    scale = np.float32(1.0 / np.sqrt(DK))
    common = {
        "wqT": np.ascontiguousarray(
            (np.asarray(w_q, np.float32).T * scale)).astype(BF),
        "wkT": np.ascontiguousarray(np.asarray(w_k, np.float32).T).astype(BF),
        "wvT": np.ascontiguousarray(np.asarray(w_v, np.float32).T).astype(BF),
        "woT": np.ascontiguousarray(np.asarray(w_o, np.float32).T).astype(BF),
        "bias": np.asarray(b_o, np.float32)[None, :].astype(BF),
    }

    in_maps = []
    for c in range(NCORES):
        b = c // 2
        chunks = CHUNKS_P[c % 2]
        xb = x[b]
        xq = np.concatenate(
            [xb[j * SC2:(j + 1) * SC2] for j in chunks], axis=0)
        in_maps.append({
            "xT": np.ascontiguousarray(xb.T).astype(BF),
            "xQT": np.ascontiguousarray(xq.T).astype(BF),
            "masks": _masks_for_core(c),
            **common,
        })

    res = run_bass_kernel_spmd(nc, in_maps, core_ids=list(range(NCORES)),
                               trace=_CACHE.get('trace', False),
                               tmpdir=_CACHE.get('tmpdir'))
    _CACHE['last_res'] = res

    y = np.empty((B, S, D), dtype=np.float32)
    for c in range(NCORES):
        b = c // 2
        chunks = CHUNKS_P[c % 2]
        yc = np.asarray(res.results[c]["y"], dtype=np.float32)
        for s, j in enumerate(chunks):
            y[b, j * SC2:(j + 1) * SC2] = yc[s * SC2:(s + 1) * SC2]
    return y


# revision 27
# speedup vs baseline: 1.3738x; 1.3738x over previous
# Multi-head causal attention (B=4, S=2048, D=1024, H=16) on 8 TRN2 NeuronCores.
#
# Sharding: batch x query-chunk. Core c handles batch b=c//2 and two 512-row
# query chunks of that batch: cores with c%2==0 take real chunks (0, 3),
# c%2==1 take (1, 2). The SPMD program is identical on every core: two query
# "slots" with fixed kk-tile capacities (8, 16); per-core causality/padding
# is expressed via multiplicative masks in input data.
#
# v3 structure: Q proj + K/V first-half run up front; K/V second-half
# projections are emitted as PE "filler" groups interleaved into the cap-8
# attention phase (their outputs are first needed at t>=8 of the cap-16
# phase). Output-projection units are interleaved at block boundaries of the
# cap-16 phase. y is written in bf16 and converted on the host.
#
#   St[kk, q]: Kt[d, s], Qt[d, q]; St = Kt_tile.T @ Qt (2 heads packed into
#   one 2-bank PSUM tile, exp'd in a single ACT op)
#   P = exp(St) * mask
#   OT[dv, q] += V_aug[kk, 65].T @ P  -- V carries a ones column, so PSUM
#     row 64 accumulates the softmax denominators for free.
import sys

if '/opt/trn_rl_repo' not in sys.path:
    sys.path.insert(0, '/opt/trn_rl_repo')

import numpy as np

B, S, D = 4, 2048, 1024
H, DK = 16, 64
NCORES = 8
SC = 512
NKT = S // 128            # 16 kk tiles
HPN = D // 128            # 8 head-pairs
CAPS = (8, 16)            # kk-tile capacity per slot (uniform across cores)
CHUNKS = [(0, 3), (1, 2)]  # real chunk pair per core parity

_CACHE = {}


def _build_program():
    import contextlib

    import concourse.tile as tile
    from concourse import bacc, mybir

    F32 = mybir.dt.float32
    BF16 = mybir.dt.bfloat16
    EXP = mybir.ActivationFunctionType.Exp

    nc = bacc.Bacc("TRN2", target_bir_lowering=False, debug=False,
                   num_devices=NCORES)

    xT_d = nc.dram_tensor("xT", [D, S], BF16, kind="ExternalInput")
    xQT_d = nc.dram_tensor("xQT", [D, 2 * SC], BF16, kind="ExternalInput")
    wqT_d = nc.dram_tensor("wqT", [D, D], BF16, kind="ExternalInput")
    wkT_d = nc.dram_tensor("wkT", [D, D], BF16, kind="ExternalInput")
    wvT_d = nc.dram_tensor("wvT", [D, D], BF16, kind="ExternalInput")
    woT_d = nc.dram_tensor("woT", [D, D], BF16, kind="ExternalInput")
    bias_d = nc.dram_tensor("bias", [1, D], BF16, kind="ExternalInput")
    masks_d = nc.dram_tensor("masks", [128, 16 * 512], BF16,
                             kind="ExternalInput")
    y_d = nc.dram_tensor("y", [2 * SC, D], BF16, kind="ExternalOutput")

    with tile.TileContext(nc) as tc, contextlib.ExitStack() as ctx:
        smalls = ctx.enter_context(tc.tile_pool(name="smalls", bufs=1))
        p_OT = ctx.enter_context(tc.tile_pool(name="otp", bufs=1))
        p_Kt = ctx.enter_context(tc.tile_pool(name="ktp", bufs=1))
        p_Qt = ctx.enter_context(tc.tile_pool(name="qtp", bufs=1))
        p_V = ctx.enter_context(tc.tile_pool(name="vp", bufs=1))
        p_mk = ctx.enter_context(tc.tile_pool(name="mk", bufs=1))
        # closed manually after the cap-8 phase to free 48 KB/partition
        proj_ctx = contextlib.ExitStack()
        p_xh1 = proj_ctx.enter_context(tc.tile_pool(name="xth1", bufs=1))
        p_wv = proj_ctx.enter_context(tc.tile_pool(name="wfv", bufs=1))
        p_wk = proj_ctx.enter_context(tc.tile_pool(name="wfk", bufs=1))

        OT = p_OT.tile([128, HPN * 2 * SC], BF16, tag="OT")
        Kt = p_Kt.tile([128, HPN * S], BF16, tag="Kt")
        Qt = p_Qt.tile([128, HPN * 2 * SC], BF16, tag="Qt")
        Vsb = p_V.tile([128, NKT * H * 65], BF16, tag="Vsb")
        masks_sb = p_mk.tile([128, 16 * 512], BF16, tag="masks")
        bias_sb = smalls.tile([1, D], BF16, tag="bias")
        ones1f = smalls.tile([1, 128], F32, tag="ones1f")
        nc.vector.memset(ones1f[:], 1.0)
        ones1 = smalls.tile([1, 128], BF16, tag="ones1")
        nc.vector.tensor_copy(ones1[:], ones1f[:])
        ones256f = smalls.tile([128, 256], F32, tag="ones256f")
        nc.vector.memset(ones256f[:], 1.0)

        # ones columns of V_aug (all 16 s-tiles, one strided copy)
        nc.vector.tensor_copy(
            Vsb[:].rearrange("p (s h c) -> p s h c", s=NKT, c=65)
            [:, :, :, 64:65],
            ones256f[:].rearrange("p (s h) -> p s h", s=NKT)[:, :, :, None])

        wv = p_wv.tile([128, 8 * D], BF16, tag="wv")
        wk = p_wk.tile([128, 8 * D], BF16, tag="wk")
        xh1 = p_xh1.tile([128, 8 * 1024], BF16, tag="xh1")
        xhs = [None, xh1]

        # ------- preamble: Q projection, then K/V first half -------------
        # xh0/wq/xq close with this scope, freeing SBUF for attention pools
        with tc.tile_pool(name="xth0", bufs=1) as p_xh0, \
             tc.tile_pool(name="wf2", bufs=1) as p_w2, \
             tc.tile_pool(name="xqs", bufs=8) as p_xq, \
             tc.tile_pool(name="psq", bufs=8, space="PSUM") as psq:
            xhs[0] = p_xh0.tile([128, 8 * 1024], BF16, tag="xh0",
                                name="xh0")
            wq = p_w2.tile([128, 8 * D], BF16, tag="w2")
            # ci-major order: a ci=1 load never queues ahead of a ci=0 load
            # it transitively depends on (buf reuse + in-order DMA queue)
            xq_tiles = {}
            for ci in range(2):
                for k in range(8):
                    if ci == 0:
                        nc.sync.dma_start(
                            wq[:, k * D:(k + 1) * D],
                            wqT_d.ap()[k * 128:(k + 1) * 128, :])
                    xq1 = p_xq.tile([128, 512], BF16, tag="xq",
                                    name=f"xq_{ci}_{k}")
                    nc.sync.dma_start(
                        xq1[:],
                        xQT_d.ap()[k * 128:(k + 1) * 128,
                                   ci * SC:(ci + 1) * SC])
                    xq_tiles[(ci, k)] = xq1
            # K/V inputs: first half of x + wk right behind, wv next,
            # second half afterwards; masks/bias late on the gpsimd queue.
            for k in range(8):
                nc.sync.dma_start(
                    xhs[0][:, k * 1024:(k + 1) * 1024],
                    xT_d.ap()[k * 128:(k + 1) * 128, 0:1024])
                nc.sync.dma_start(
                    wk[:, k * D:(k + 1) * D],
                    wkT_d.ap()[k * 128:(k + 1) * 128, :])
            for k in range(8):
                nc.sync.dma_start(
                    wv[:, k * D:(k + 1) * D],
                    wvT_d.ap()[k * 128:(k + 1) * 128, :])
                nc.sync.dma_start(
                    xhs[1][:, k * 1024:(k + 1) * 1024],
                    xT_d.ap()[k * 128:(k + 1) * 128, 1024:2048])
            nc.gpsimd.dma_start(masks_sb[:], masks_d.ap())
            nc.gpsimd.dma_start(bias_sb[:], bias_d.ap())

            for ci in range(2):
                ps8 = [psq.tile([128, 512], F32, tag="ps",
                                name=f"psq_{ci}_{hp}") for hp in range(HPN)]
                for k in range(8):
                    for hp in range(HPN):
                        nc.tensor.matmul(
                            ps8[hp][:],
                            wq[:, k * D + hp * 128:k * D + (hp + 1) * 128],
                            xq_tiles[(ci, k)][:],
                            start=(k == 0), stop=(k == 7))
                for hp in range(HPN):
                    nc.vector.tensor_copy(
                        Qt[:, hp * 2 * SC + ci * SC:
                           hp * 2 * SC + (ci + 1) * SC],
                        ps8[hp][:])

            # --------- K + V projections for the first sequence half -----
            xh = xhs[0]
            for sc2 in range(2):
                ps8 = [psq.tile([128, 512], F32, tag="ps",
                                name=f"psk_{sc2}_{hp}")
                       for hp in range(HPN)]
                for k in range(8):
                    for hp in range(HPN):
                        nc.tensor.matmul(
                            ps8[hp][:],
                            wk[:, k * D + hp * 128:k * D + (hp + 1) * 128],
                            xh[:, k * 1024 + sc2 * 512:
                               k * 1024 + (sc2 + 1) * 512],
                            start=(k == 0), stop=(k == 7))
                for hp in range(HPN):
                    nc.vector.tensor_copy(
                        Kt[:, hp * S + sc2 * 512:hp * S + (sc2 + 1) * 512],
                        ps8[hp][:])
            for sti in range(8):
                for dvc in range(2):
                    ps = psq.tile([128, 512], F32, tag="ps")
                    for k in range(8):
                        nc.tensor.matmul(
                            ps[:],
                            xh[:, k * 1024 + sti * 128:
                               k * 1024 + (sti + 1) * 128],
                            wv[:, k * D + dvc * 512:k * D + (dvc + 1) * 512],
                            start=(k == 0), stop=(k == 7))
                    off = sti * 1040 + dvc * 520
                    nc.vector.tensor_copy(
                        Vsb[:, off:off + 520]
                        .rearrange("p (h c) -> p h c", c=65)[:, :, 0:64],
                        ps[:].rearrange("p (h c) -> p h c", c=64))

        # ---------------- attention with interleaved fillers -------------
        # (rs/bc/P pools are per-phase so pool closes stay LIFO-ordered)
        with contextlib.nullcontext():

            # ---- filler generators: K/V projections for sequence half 1,
            # each a closure emitting ~1.8us of PE work into pool `fx`.
            def kh1_filler(sc, hp, fx):
                def emit():
                    xh = xhs[1]
                    sc2 = sc - 2
                    ps = fx.tile([128, 512], F32, tag="fx",
                                 name=f"fk_{sc}_{hp}")
                    for k in range(8):
                        nc.tensor.matmul(
                            ps[:],
                            wk[:, k * D + hp * 128:k * D + (hp + 1) * 128],
                            xh[:, k * 1024 + sc2 * 512:
                               k * 1024 + (sc2 + 1) * 512],
                            start=(k == 0), stop=(k == 7))
                    nc.vector.tensor_copy(
                        Kt[:, hp * S + sc * 512:hp * S + (sc + 1) * 512],
                        ps[:])
                return emit

            def vh1_filler(sti, dvc, fx):
                def emit():
                    xh = xhs[1]
                    st_g = 8 + sti
                    ps = fx.tile([128, 512], F32, tag="fx",
                                 name=f"fv_{sti}_{dvc}")
                    for k in range(8):
                        nc.tensor.matmul(
                            ps[:],
                            xh[:, k * 1024 + sti * 128:
                               k * 1024 + (sti + 1) * 128],
                            wv[:, k * D + dvc * 512:k * D + (dvc + 1) * 512],
                            start=(k == 0), stop=(k == 7))
                    off = st_g * 1040 + dvc * 520
                    nc.vector.tensor_copy(
                        Vsb[:, off:off + 520]
                        .rearrange("p (h c) -> p h c", c=65)[:, :, 0:64],
                        ps[:].rearrange("p (h c) -> p h c", c=64))
                return emit

            def outproj_unit(qi, nc2, pool, wo, p_yb):
                def emit():
                    ps = pool.tile([128, 512], F32, tag="av",
                                   name=f"psy_{qi}_{nc2}")
                    for dc in range(8):
                        nc.tensor.matmul(
                            ps[:],
                            OT[:, dc * 2 * SC + qi * 128:
                               dc * 2 * SC + (qi + 1) * 128],
                            wo[:, dc * D + nc2 * 512:
                               dc * D + (nc2 + 1) * 512],
                            start=(dc == 0), stop=False)
                    nc.tensor.matmul(
                        ps[:], ones1[:],
                        bias_sb[0:1, nc2 * 512:(nc2 + 1) * 512],
                        start=False, stop=True)
                    yb = p_yb.tile([128, 512], BF16, tag="yb")
                    nc.vector.tensor_copy(yb[:], ps[:])
                    nc.sync.dma_start(
                        y_d.ap()[qi * 128:(qi + 1) * 128,
                                 nc2 * 512:(nc2 + 1) * 512], yb[:])
                return emit

            def attn_block(ci, bl, cap, p_st, p_av, fillers,
                           p_P, p_rs, p_bc):
                av = [p_av.tile([128, 512], F32, tag="av",
                                name=f"av_{ci}_{bl}_{i}")
                      for i in range(4)]

                def emit_av(t, p_tiles):
                    for hp_i in range(2):
                        for hh in range(2):
                            hi = 2 * hp_i + hh
                            off = (t * 1040 + (2 * bl + hp_i) * 130 +
                                   hh * 65)
                            nc.tensor.matmul(
                                av[hi][0:65, :],
                                Vsb[:, off:off + 65],
                                p_tiles[hp_i][:, hh * 512:(hh + 1) * 512],
                                start=(t == 0), stop=(t == cap - 1))

                def emit_scores_exp(t, hp_i, p_cur):
                    hp = 2 * bl + hp_i
                    st = p_st.tile([128, 1024], F32, tag="st")
                    for hh in range(2):
                        r0 = 64 * hh
                        nc.tensor.matmul(
                            st[:, hh * 512:(hh + 1) * 512],
                            Kt[r0:r0 + 64,
                               hp * S + t * 128:hp * S + (t + 1) * 128],
                            Qt[r0:r0 + 64,
                               hp * 2 * SC + ci * SC:
                               hp * 2 * SC + (ci + 1) * SC],
                            start=True, stop=True,
                            tile_position=(r0, 0))
                    p1 = p_P.tile([128, 1024], BF16, tag="p")
                    nc.scalar.activation(p1[:], st[:], EXP)
                    if ci == 0 or t >= 8:
                        midx = t if ci == 0 else 8 + (t - 8)
                        p2 = p_P.tile([128, 1024], BF16, tag="p")
                        for hf in range(2):
                            nc.vector.tensor_mul(
                                p2[:, hf * 512:(hf + 1) * 512],
                                p1[:, hf * 512:(hf + 1) * 512],
                                masks_sb[:, midx * 512:(midx + 1) * 512])
                        p1 = p2
                    p_cur.append(p1)

                # lag-2 software pipeline; AV + filler PE work sits between
                # the two score groups so the st-pool WAR dependency on
                # exp(hp_i=0) never stalls the PE queue head.
                pending = []
                for t in range(cap):
                    p_cur = []
                    emit_scores_exp(t, 0, p_cur)
                    if len(pending) > 2:
                        tt, pp_t = pending.pop(0)
                        emit_av(tt, pp_t)
                    if fillers:
                        fillers.pop(0)()
                    emit_scores_exp(t, 1, p_cur)
                    pending.append((t, p_cur))
                for tt, pp_t in pending:
                    emit_av(tt, pp_t)
                # normalize, one head-pair at a time: reciprocal on the
                # [1,1024] denominator row, then broadcast the reciprocals
                for hp_i in range(2):
                    hp = 2 * bl + hp_i
                    rs = p_rs.tile([1, 1024], F32, tag="rs")
                    for hh in range(2):
                        hi = 2 * hp_i + hh
                        nc.vector.tensor_copy(
                            rs[0:1, hh * 512:hh * 512 + 512],
                            av[hi][64:65, :])
                    rrs = p_rs.tile([1, 1024], F32, tag="rrs")
                    scr = p_rs.tile([1, 1024], F32, tag="scr")
                    nc.vector.reciprocal_approx_accurate(
                        rrs[:], rs[:], scratch=scr[:])
                    rbc = p_bc.tile([128, 1024], F32, tag="rbc")
                    nc.gpsimd.partition_broadcast(rbc[:], rrs[:])
                    for hh in range(2):
                        hi = 2 * hp_i + hh
                        r0 = 64 * hh
                        nc.vector.tensor_mul(
                            OT[r0:r0 + 64,
                               hp * 2 * SC + ci * SC:
                               hp * 2 * SC + (ci + 1) * SC],
                            av[hi][0:64, :],
                            rbc[r0:r0 + 64, hh * 512:hh * 512 + 512])

            # ---- cap-8 phase: st x1 + av x4 + flex x2 banks; K/V second
            # half drains through the flex pool as filler work.
            with tc.tile_pool(name="rs0", bufs=1) as p_rs0, \
                 tc.tile_pool(name="bcp0", bufs=1) as p_bc0, \
                 tc.tile_pool(name="pp0", bufs=8) as p_P0, \
                 tc.tile_pool(name="pst0", bufs=1, space="PSUM") as p_st0, \
                 tc.tile_pool(name="pav0", bufs=4, space="PSUM") as p_av0, \
                 tc.tile_pool(name="pfx", bufs=2, space="PSUM") as p_fx:
                fillers = []
                for hp in range(HPN):
                    fillers.append(kh1_filler(2, hp, p_fx))
                for sti in range(4):
                    for dvc in range(2):
                        fillers.append(vh1_filler(sti, dvc, p_fx))
                for hp in range(HPN):
                    fillers.append(kh1_filler(3, hp, p_fx))
                for sti in range(4, 8):
                    for dvc in range(2):
                        fillers.append(vh1_filler(sti, dvc, p_fx))
                for bl in range(HPN // 2):
                    attn_block(0, bl, CAPS[0], p_st0, p_av0, fillers,
                               p_P0, p_rs0, p_bc0)
                while fillers:
                    fillers.pop(0)()

            proj_ctx.close()  # free xh1/wv/wk SBUF for wo/yb

            # ---- cap-16 phase: st x2 + av x4; out-proj units for the
            # finished cap-8 slot interleave at block boundaries.
            with tc.tile_pool(name="wo", bufs=1) as p_wo, \
                 tc.tile_pool(name="ybp", bufs=4) as p_yb, \
                 tc.tile_pool(name="rs1", bufs=1) as p_rs1, \
                 tc.tile_pool(name="bcp1", bufs=1) as p_bc1, \
                 tc.tile_pool(name="pp1", bufs=8) as p_P1, \
                 tc.tile_pool(name="pst1", bufs=2, space="PSUM") as p_st1, \
                 tc.tile_pool(name="pav1", bufs=4, space="PSUM") as p_av1:
                wo = p_wo.tile([128, 8 * D], BF16, tag="wo")
                for k in range(8):
                    nc.sync.dma_start(
                        wo[:, k * D:(k + 1) * D],
                        woT_d.ap()[k * 128:(k + 1) * 128, :])
                outp = [outproj_unit(qi, nc2, p_av1, wo, p_yb)
                        for qi in range(4) for nc2 in range(2)]
                for bl in range(HPN // 2):
                    attn_block(1, bl, CAPS[1], p_st1, p_av1, [],
                               p_P1, p_rs1, p_bc1)
                    for _ in range(2):
                        if outp:
                            outp.pop(0)()
                # remaining out-proj: the cap-16 slot's queries
                for qi in range(4, 8):
                    for nc2 in range(2):
                        outproj_unit(qi, nc2, p_av1, wo, p_yb)()

    nc.compile()
    return nc


def _get_program():
    if 'nc' not in _CACHE:
        _CACHE['nc'] = _build_program()
    return _CACHE['nc']


def _tri_masks():
    p = np.arange(128)[:, None]
    f = np.arange(SC)[None, :]
    return [(p <= f - 128 * r).astype(np.float32) for r in range(4)]


def _masks_for_core(c):
    """Multiplicative masks [128, 16*512]: slot t<8 serves the cap-8 slot
    (chunk j1), t>=8 the cap-16 slot (chunk j2, applied at t>=8 only)."""
    import ml_dtypes
    tri = _tri_masks()
    ones = np.ones((128, SC), np.float32)
    zeros = np.zeros((128, SC), np.float32)
    j_pair = CHUNKS[c % 2]
    out = np.zeros((128, 16 * 512), np.float32)
    for ci, cap in enumerate(CAPS):
        j = j_pair[ci]
        t0 = 0 if ci == 0 else 8
        for t in range(t0, cap):
            if t < 4 * j:
                m = ones
            elif t < 4 * j + 4:
                m = tri[t - 4 * j]
            else:
                m = zeros
            out[:, t * 512:(t + 1) * 512] = m
    return out.astype(ml_dtypes.bfloat16)


def kernel(x, w_q, w_k, w_v, w_o, b_o):
    import ml_dtypes
    from concourse.bass_utils import run_bass_kernel_spmd

    BF = ml_dtypes.bfloat16
    x = np.asarray(x, dtype=np.float32)
    nc = _get_program()

    scale = np.float32(1.0 / np.sqrt(DK))
    common = {
        "wqT": np.ascontiguousarray(
            (np.asarray(w_q, np.float32).T * scale)).astype(BF),
        "wkT": np.ascontiguousarray(np.asarray(w_k, np.float32).T).astype(BF),
        "wvT": np.ascontiguousarray(np.asarray(w_v, np.float32).T).astype(BF),
        "woT": np.ascontiguousarray(np.asarray(w_o, np.float32).T).astype(BF),
        "bias": np.asarray(b_o, np.float32)[None, :].astype(BF),
    }

    in_maps = []
    for c in range(NCORES):
        b = c // 2
        j1, j2 = CHUNKS[c % 2]
        xb = x[b]
        xq = np.concatenate(
            [xb[j1 * SC:(j1 + 1) * SC], xb[j2 * SC:(j2 + 1) * SC]], axis=0)
        in_maps.append({
            "xT": np.ascontiguousarray(xb.T).astype(BF),
            "xQT": np.ascontiguousarray(xq.T).astype(BF),
            "masks": _masks_for_core(c),
            **common,
        })

    res = run_bass_kernel_spmd(nc, in_maps, core_ids=list(range(NCORES)),
                               trace=_CACHE.get('trace', False),
                               tmpdir=_CACHE.get('tmpdir'))
    _CACHE['last_res'] = res

    y = np.empty((B, S, D), dtype=np.float32)
    for c in range(NCORES):
        b = c // 2
        j1, j2 = CHUNKS[c % 2]
        yc = np.asarray(res.results[c]["y"], dtype=np.float32)
        y[b, j1 * SC:(j1 + 1) * SC] = yc[0:SC]
        y[b, j2 * SC:(j2 + 1) * SC] = yc[SC:2 * SC]
    return y
